# revision 5
# baseline (speedup 1.0000x reference)
"""Trainium2 Bass kernel for nn_Agent_56899726737926 (segment_reduce).

Shapes (hardcoded to the spec):
  logits [1e6, 8] f32, edge_vf [4e6, 8] f32, node_batch [1e6] i32,
  entry_type/entry_id/entry_loc [2097152] i32 (entry_loc sorted),
  loc_graph [262144] i32, action_loc [64] i32  ->  out [2, 64] f32

Two SPMD launches on 8 NeuronCores:
  Phase 1: dense row-sums of logits and edge_vf[:1M] (entries only ever
           reference ids < 1e6) -> per-id score table.
  Phase 2: per-slot table gather + segmented (per-location) sums via a
           (graph, loc)-aligned slot grid + per-partition online-softmax
           stats; host combines [128,4] per-core stats into the output.

Structural assumptions are checked at runtime; any violation falls back to
an exact numpy implementation.
"""
import os
import numpy as np

import walrus_flags
walrus_flags.enable(["--dge-levels=vector_dynamic_offsets"])

from concourse.bass_utils import run_bass_kernel_spmd  # noqa: E402
from kern_phase1 import build_phase1  # noqa: E402
from kern_phase2b import build_phase2b  # noqa: E402

# NOTE: before submission the sibling modules (walrus_flags, wait_split,
# kern_phase1, kern_phase2b) get inlined here so kernel.py is self-contained.

P = 128
NCORES = 8
N = 1_000_000
F = 8
L = 262_144
NE = 2_097_152
B = 64

R1 = 977                      # rows per partition in phase 1
SH = P * R1                   # 125056 rows per core (last shard padded)

ZERO_KEY = 2_000_000
TPAD = 2_000_128

WTARGET = 2176                # per-partition fill threshold (slots)
W = 2304                      # per-partition slot capacity
MAXLOC = 126                  # max entries of one loc the grid tolerates

GATHER_MECH = os.environ.get("KERNEL_GATHER_MECH", "rowchain")
VERBOSE = os.environ.get("KERNEL_VERBOSE", "0") == "1"

_cache = {}


def _run_spmd(nc, in_maps):
    import time
    t0 = time.time()
    r = run_bass_kernel_spmd(nc, in_maps, list(range(len(in_maps))),
                             trace=False)
    if VERBOSE:
        print(f"[kernel] spmd launch wall={time.time()-t0:.3f}s", flush=True)
    return r.results


def _ref_numpy(logits, edge_vf, node_batch, entry_type, entry_id, entry_loc,
               loc_graph, action_loc):
    """Exact numpy port of the reference (fallback path)."""
    n_loc = loc_graph.shape[0]
    n_graph = action_loc.shape[0]
    node_val = logits[entry_id].sum(-1)
    edge_val = edge_vf[entry_id].sum(-1)
    vals = np.where(entry_type == 1, node_val, edge_val).astype(np.float64)
    loc_scores = np.zeros(n_loc, np.float64)
    np.add.at(loc_scores, entry_loc, vals)
    counts = np.bincount(node_batch, minlength=n_graph).astype(np.float64)
    g_sum = np.zeros((n_graph, logits.shape[1]), np.float64)
    np.add.at(g_sum, node_batch, logits.astype(np.float64))
    m = (g_sum / np.maximum(counts, 1.0)[:, None]).mean(-1)
    seg_max = np.full(n_graph, -np.inf)
    np.maximum.at(seg_max, loc_graph, loc_scores)
    M = np.maximum(seg_max, m)
    ex = np.exp(loc_scores - M[loc_graph])
    em = np.exp(m - M)
    Z = np.zeros(n_graph, np.float64)
    np.add.at(Z, loc_graph, ex)
    Z += em
    lse = np.log(Z) + M
    ps = np.zeros(n_graph, np.float64)
    np.add.at(ps, loc_graph, loc_scores * ex)
    ps += m * em
    entropy = lse - ps / Z
    g = loc_graph[action_loc]
    log_probs = loc_scores[action_loc] - lse[g]
    return np.stack([log_probs, entropy]).astype(np.float32)


def _get_nc(name):
    if name in _cache:
        return _cache[name]
    if name == "phase1":
        nc = build_phase1(R1, n_chunks=4)
    elif name.startswith("phase2"):
        nc = build_phase2b(W, mech=name.split(":")[1], tpad=TPAD)
    _cache[name] = nc
    return nc


def _pad_shards(arr):
    """arr [N, F] -> 8 contiguous shards [SH, F] (last one zero-padded)."""
    shards = []
    for c in range(NCORES):
        lo, hi = SH * c, SH * (c + 1)
        if hi <= arr.shape[0]:
            shards.append(arr[lo:hi])
        else:
            pad = np.zeros((hi - arr.shape[0], arr.shape[1]), arr.dtype)
            shards.append(np.ascontiguousarray(
                np.concatenate([arr[lo:], pad], axis=0)))
    return shards


def kernel(**inputs):
    logits = np.asarray(inputs["logits"], np.float32)
    edge_vf = np.asarray(inputs["edge_vf"], np.float32)
    node_batch = np.asarray(inputs["node_batch"], np.int32)
    entry_type = np.asarray(inputs["entry_type"], np.int32)
    entry_id = np.asarray(inputs["entry_id"], np.int32)
    entry_loc = np.asarray(inputs["entry_loc"], np.int32)
    loc_graph = np.asarray(inputs["loc_graph"], np.int32)
    action_loc = np.asarray(inputs["action_loc"], np.int32)

    def fallback(reason):
        if VERBOSE:
            print(f"[kernel] FALLBACK: {reason}", flush=True)
        return _ref_numpy(logits, edge_vf, node_batch, entry_type, entry_id,
                          entry_loc, loc_graph, action_loc)

    # ---- structural checks ----
    if (logits.shape != (N, F) or edge_vf.shape[1] != F
            or entry_loc.shape != (NE,) or loc_graph.shape != (L,)
            or action_loc.shape != (B,)):
        return fallback("shape")
    if entry_id.min() < 0 or entry_id.max() >= N:
        return fallback("entry_id range")
    if np.any(np.diff(entry_loc) < 0):
        return fallback("entry_loc not sorted")
    if entry_loc.min() < 0 or entry_loc.max() >= L:
        return fallback("entry_loc range")
    if loc_graph.min() < 0 or loc_graph.max() >= B:
        return fallback("loc_graph range")
    if node_batch.min() < 0 or node_batch.max() >= B:
        return fallback("node_batch range")
    if action_loc.min() < 0 or action_loc.max() >= L:
        return fallback("action_loc range")

    # ---- phase 1: row sums on device ----
    lg_sh = _pad_shards(logits)
    ed_sh = _pad_shards(edge_vf[:N])
    in_maps1 = [{"lg": lg_sh[c], "ed": ed_sh[c]} for c in range(NCORES)]
    r1 = _run_spmd(_get_nc("phase1"), in_maps1)
    node_sum = np.concatenate([r1[c]["ns"] for c in range(NCORES)])[:N]
    edge_sum = np.concatenate([r1[c]["es"] for c in range(NCORES)])[:N]

    table = np.zeros(TPAD, np.float32)
    table[0:N] = edge_sum
    table[N:2 * N] = node_sum

    counts = np.bincount(node_batch, minlength=B).astype(np.float64)
    msum = np.bincount(node_batch, weights=node_sum.astype(np.float64),
                       minlength=B)
    m = (msum / F) / np.maximum(counts, 1.0)

    key = (entry_id + N * entry_type).astype(np.int32)

    # ---- slot grid construction (host, index metadata only) ----
    cnt = np.bincount(entry_loc, minlength=L).astype(np.int64)
    if cnt.max() > MAXLOC:
        return fallback("loc too big")
    nz = np.flatnonzero(cnt)                      # non-empty locs only
    g_nz = loc_graph[nz].astype(np.int64)
    s_nz = cnt[nz]
    order = np.argsort(g_nz, kind="stable")       # group locs by graph
    locs_o = nz[order]
    g_o = g_nz[order]
    s_o = s_nz[order]
    css = np.cumsum(s_o)
    start = css - s_o
    gslots = np.bincount(g_o, weights=s_o, minlength=B).astype(np.int64)
    gbase = np.concatenate([[0], np.cumsum(gslots)[:-1]])
    start_in_g = start - gbase[g_o]
    if gslots.max() > 16 * WTARGET:
        return fallback("graph capacity")
    p_loc = start_in_g // WTARGET                 # partition within graph
    pairkey = g_o * 16 + p_loc                    # nondecreasing
    uniq, first_idx = np.unique(pairkey, return_index=True)
    pair_base = np.zeros(B * 16, np.int64)
    pair_base[uniq] = start_in_g[first_idx]
    col_o = start_in_g - pair_base[pairkey]
    if (col_o + s_o).max() > W:
        return fallback("partition capacity")

    # map back to loc-id indexing (non-empty locs)
    col_of_loc = np.zeros(L, np.int64)
    part_of_loc = np.zeros(L, np.int64)
    core_of_loc = np.zeros(L, np.int64)
    col_of_loc[locs_o] = col_o
    part_of_loc[locs_o] = 16 * (g_o % 8) + p_loc
    core_of_loc[locs_o] = g_o // 8

    # per-entry slot
    loc_entry_start = np.concatenate([[0], np.cumsum(cnt)[:-1]])
    rank = np.arange(NE, dtype=np.int64) - loc_entry_start[entry_loc]
    e_core = core_of_loc[entry_loc]
    e_part = part_of_loc[entry_loc]
    e_col = col_of_loc[entry_loc] + rank

    keys_grid = np.full((NCORES, P, W), ZERO_KEY, np.int32)
    keys_grid[e_core, e_part, e_col] = key
    # masks: bit0 = continuation flag (0 at loc start), bit1 = loc end,
    # bit2 = action end
    masks = np.ones((NCORES, P, W), np.int8)
    c_l = core_of_loc[locs_o]
    p_l = part_of_loc[locs_o]
    masks[c_l, p_l, col_o] = 0                                # loc starts
    np.bitwise_or.at(masks, (c_l, p_l, col_o + s_o - 1), 2)   # loc ends

    al = action_loc.astype(np.int64)
    g_act = loc_graph[al].astype(np.int64)
    if len(np.unique(g_act)) != B:
        return fallback("action graph collision")
    al_nz = al[cnt[al] > 0]
    a_core = core_of_loc[al_nz]
    a_part = part_of_loc[al_nz]
    a_col = col_of_loc[al_nz] + cnt[al_nz] - 1
    np.bitwise_or.at(masks, (a_core, a_part, a_col), 4)

    # ---- phase 2 ----
    mech = GATHER_MECH
    in_maps2 = []
    for c in range(NCORES):
        im = {"table": table.reshape(TPAD, 1),
              "keys": keys_grid[c], "masks": masks[c]}
        if mech == "hostgather":
            im["vals_in"] = table[keys_grid[c]]
        in_maps2.append(im)
    r2 = _run_spmd(_get_nc(f"phase2:{mech}"), in_maps2)
    stats = np.stack([r2[c]["stats"] for c in range(NCORES)])

    # ---- host combine ----
    Mp = stats[:, :, 0].astype(np.float64).reshape(B, 16)
    Zp = stats[:, :, 1].astype(np.float64).reshape(B, 16)
    Sp = stats[:, :, 2].astype(np.float64).reshape(B, 16)
    act = stats[:, :, 3].astype(np.float64).reshape(B, 16)

    n_empty = np.bincount(loc_graph[cnt == 0], minlength=B).astype(np.float64)
    Mg = np.maximum(Mp.max(axis=1), m)
    Mg = np.where(n_empty > 0, np.maximum(Mg, 0.0), Mg)
    scale = np.exp(np.clip(Mp - Mg[:, None], -745, 0))
    em = np.exp(m - Mg)
    Z = (Zp * scale).sum(1) + em + n_empty * np.exp(-Mg)
    S = (Sp * scale).sum(1) + m * em
    lse = np.log(Z) + Mg
    entropy = lse - S / Z

    act_by_graph = act.sum(1)
    score_b = np.where(cnt[al] > 0, act_by_graph[g_act], 0.0)
    log_probs = score_b - lse[g_act]
    return np.stack([log_probs, entropy]).astype(np.float32)
